# revision 1
# baseline (speedup 1.0000x reference)
"""GNN sparse-attention message passing on 8 Trainium2 NeuronCores.

Strategy (edge parallelism, sharded by destination node):
- Sort edges by dst; split nodes into 8 contiguous ranges with ~equal edge counts.
- Per core, pack edges into groups of G tiles x 128 edges; each group's dst nodes
  lie in a window of <=128 consecutive node ids (dst_local = dst - group_base).
- Per tile: gather k|v rows (combined 256-col table) and q rows per edge via
  indirect DMA; score = exp(clip(sum_d k*q / 4)); msg = v * score.
- One-hot matmul (S_T[e, n] = dst_local[e]==n) accumulates [wV | Z] for the
  group's window in PSUM across the group's tiles; divide and indirect-scatter
  the 128 window rows to the per-core output slab; host concatenates slabs.
"""
import math
import numpy as np

import concourse.bass as bass
import concourse.tile as tile
from concourse import bacc, mybir
from concourse.bass_utils import run_bass_kernel_spmd

N = 50000
E = 800000
HID = 128
HEADS = 8
HD = 16
NCORES = 8
G = 12            # tiles per group
P = 128
CLIP_LO = float(np.exp(-5.0))
CLIP_HI = float(np.exp(5.0))

_cache = {}


def _pack(e_src, e_dst):
    """Sort edges by dst, shard across cores, pack into groups/tiles.

    Returns per-core arrays + layout info. All cores padded to the same
    group count Gmax and out-slab size MAXN+128.
    """
    order = np.argsort(e_dst, kind="stable")
    s = e_src[order].astype(np.int64)
    d = e_dst[order].astype(np.int64)
    deg = np.bincount(d, minlength=N)
    cum = np.cumsum(deg)
    # core boundaries in node space, ~equal edges
    bounds = [0]
    for c in range(1, NCORES):
        target = E * c // NCORES
        bounds.append(int(np.searchsorted(cum, target)))
    bounds.append(N)

    cores = []
    for c in range(NCORES):
        n0, n1 = bounds[c], bounds[c + 1]
        e0 = 0 if n0 == 0 else int(cum[n0 - 1])
        e1 = int(cum[n1 - 1]) if n1 > 0 else 0
        cs, cd = s[e0:e1], d[e0:e1]
        nodes = np.arange(n0, n1)
        ndeg = deg[n0:n1]
        groups = []   # (base, [edge index ranges]) per group
        ei = 0        # edge cursor within this core
        ni = 0        # node cursor within range
        while ni < len(nodes):
            base = int(nodes[ni])
            used = 0
            cap = G * P
            gstart = ei
            while ni < len(nodes):
                node = int(nodes[ni])
                dg = int(ndeg[ni])
                if node - base >= P:
                    break
                if used + dg > cap:
                    break
                used += dg
                ei += dg
                ni += 1
            groups.append((base, gstart, ei))
        cores.append((n0, n1, cs, cd, groups))

    Gmax = max(len(cr[4]) for cr in cores)
    MAXN = max(cr[1] - cr[0] for cr in cores)
    MAXN = ((MAXN + 127) // 128) * 128

    per_core = []
    for (n0, n1, cs, cd, groups) in cores:
        ng = len(groups)
        meta = np.zeros((Gmax, 15, P), np.int32)       # [g, col, p]
        dstl = np.full((Gmax, G, P), -1.0, np.float32)  # local dst or -1
        dstg = np.zeros((Gmax, G, P), np.int32)         # per-edge global dst (for q)
        trash = MAXN + np.arange(P, dtype=np.int32)
        for g in range(Gmax):
            if g < ng:
                base, ea, eb = groups[g]
                nxt = groups[g + 1][0] if g + 1 < ng else n1
                span = min(nxt - base, P)
                r = np.arange(P)
                meta[g, 12] = np.minimum(base + r, N - 1)           # qrow (unused now)
                meta[g, 13] = np.where(r < span, (base - n0) + r, trash)  # out rows
                es, ed = cs[ea:eb], cd[ea:eb]
                ne = eb - ea
                for t in range(G):
                    lo, hi = t * P, min((t + 1) * P, ne)
                    if lo >= ne:
                        break
                    k = hi - lo
                    meta[g, t, :k] = es[lo:hi]
                    dstl[g, t, :k] = (ed[lo:hi] - base).astype(np.float32)
                    dstg[g, t, :k] = ed[lo:hi]
            else:
                meta[g, 13] = trash
        # transpose to [P, ...] SBUF-friendly layouts
        per_core.append({
            "meta": np.ascontiguousarray(meta.transpose(2, 0, 1)).reshape(P, Gmax * 15),
            "dstl": np.ascontiguousarray(dstl.transpose(2, 0, 1)).reshape(P, Gmax * G),
            "dstg": np.ascontiguousarray(dstg.transpose(2, 0, 1)).reshape(P, Gmax * G),
            "n0": n0, "n1": n1,
        })
    return per_core, Gmax, MAXN


def _build(Gmax, MAXN):
    nc = bacc.Bacc(None, target_bir_lowering=False)
    kv = nc.declare_dram_parameter("kv", [N, 2 * HID], mybir.dt.float32, isOutput=False)
    qt = nc.declare_dram_parameter("qt", [N, HID], mybir.dt.float32, isOutput=False)
    meta = nc.declare_dram_parameter("meta", [P, Gmax * 15], mybir.dt.int32, isOutput=False)
    dstl = nc.declare_dram_parameter("dstl", [P, Gmax * G], mybir.dt.float32, isOutput=False)
    dstg = nc.declare_dram_parameter("dstg", [P, Gmax * G], mybir.dt.int32, isOutput=False)
    xout = nc.declare_dram_parameter("xout", [MAXN + P, HID], mybir.dt.float32, isOutput=True)

    f32 = mybir.dt.float32
    with tile.TileContext(nc) as tc:
        with tc.tile_pool(name="const", bufs=1) as cp, \
             tc.tile_pool(name="sbuf", bufs=3) as sb, \
             tc.tile_pool(name="meta", bufs=2) as mp, \
             tc.tile_pool(name="psum", bufs=2, space="PSUM") as ps:
            ii = cp.tile([P, P], mybir.dt.int32)
            nc.gpsimd.iota(ii[:], pattern=[[1, P]], base=0, channel_multiplier=0)
            fiota = cp.tile([P, P], f32)
            nc.vector.tensor_copy(out=fiota[:], in_=ii[:])

            for g in range(Gmax):
                meta_sb = mp.tile([P, 15], mybir.dt.int32, tag="meta")
                nc.sync.dma_start(out=meta_sb[:], in_=meta[:, g * 15:(g + 1) * 15])
                dstl_sb = mp.tile([P, G], f32, tag="dstl")
                nc.sync.dma_start(out=dstl_sb[:], in_=dstl[:, g * G:(g + 1) * G])
                dstg_sb = mp.tile([P, G], mybir.dt.int32, tag="dstg")
                nc.sync.dma_start(out=dstg_sb[:], in_=dstg[:, g * G:(g + 1) * G])

                acc = ps.tile([P, HID + HEADS], f32, space="PSUM", tag="acc")
                for t in range(G):
                    kvt = sb.tile([P, 2 * HID], f32, tag="kv")
                    nc.gpsimd.indirect_dma_start(
                        out=kvt[:], out_offset=None, in_=kv[:],
                        in_offset=bass.IndirectOffsetOnAxis(ap=meta_sb[:, t:t + 1], axis=0))
                    qe = sb.tile([P, HID], f32, tag="qe")
                    nc.gpsimd.indirect_dma_start(
                        out=qe[:], out_offset=None, in_=qt[:],
                        in_offset=bass.IndirectOffsetOnAxis(ap=dstg_sb[:, t:t + 1], axis=0))

                    st = sb.tile([P, P], f32, tag="st")
                    nc.vector.tensor_tensor(
                        out=st[:], in0=dstl_sb[:, t:t + 1].to_broadcast([P, P]),
                        in1=fiota[:], op=mybir.AluOpType.is_equal)

                    prod = sb.tile([P, HID], f32, tag="prod")
                    nc.vector.tensor_tensor(
                        out=prod[:], in0=kvt[:, :HID], in1=qe[:],
                        op=mybir.AluOpType.mult)
                    sc = sb.tile([P, HEADS], f32, tag="sc")
                    nc.vector.tensor_reduce(
                        out=sc[:], in_=prod[:].rearrange("p (h d) -> p h d", h=HEADS),
                        axis=mybir.AxisListType.X, op=mybir.AluOpType.add)
                    nc.scalar.activation(
                        out=sc[:], in_=sc[:],
                        func=mybir.ActivationFunctionType.Exp, scale=1.0 / math.sqrt(HD))
                    msgext = sb.tile([P, HID + HEADS], f32, tag="msgext")
                    nc.vector.tensor_scalar(
                        out=msgext[:, HID:], in0=sc[:],
                        scalar1=CLIP_LO, scalar2=CLIP_HI,
                        op0=mybir.AluOpType.max, op1=mybir.AluOpType.min)
                    nc.vector.tensor_tensor(
                        out=msgext[:, :HID].rearrange("p (h d) -> p h d", h=HEADS),
                        in0=kvt[:, HID:].rearrange("p (h d) -> p h d", h=HEADS),
                        in1=msgext[:, HID:][:, :, None].to_broadcast([P, HEADS, HD]),
                        op=mybir.AluOpType.mult)
                    nc.tensor.matmul(out=acc[:], lhsT=st[:], rhs=msgext[:],
                                     start=(t == 0), stop=(t == G - 1))

                zr = sb.tile([P, HEADS], f32, tag="zr")
                nc.vector.tensor_scalar(out=zr[:], in0=acc[:, HID:], scalar1=1e-6,
                                        scalar2=None, op0=mybir.AluOpType.add)
                nc.vector.reciprocal(out=zr[:], in_=zr[:])
                xsb = sb.tile([P, HID], f32, tag="xsb")
                nc.vector.tensor_tensor(
                    out=xsb[:].rearrange("p (h d) -> p h d", h=HEADS),
                    in0=acc[:, :HID].rearrange("p (h d) -> p h d", h=HEADS),
                    in1=zr[:][:, :, None].to_broadcast([P, HEADS, HD]),
                    op=mybir.AluOpType.mult)
                nc.gpsimd.indirect_dma_start(
                    out=xout[:], out_offset=bass.IndirectOffsetOnAxis(
                        ap=meta_sb[:, 13:14], axis=0),
                    in_=xsb[:], in_offset=None)
    nc.finalize()
    return nc


def kernel(q, k, v, edge_index):
    q = np.asarray(q, np.float32).reshape(N, HID)
    k = np.asarray(k, np.float32).reshape(N, HID)
    v = np.asarray(v, np.float32).reshape(N, HID)
    e = np.asarray(edge_index)
    per_core, Gmax, MAXN = _pack(e[0].astype(np.int64), e[1].astype(np.int64))

    key = (Gmax, MAXN)
    if key not in _cache:
        _cache[key] = _build(Gmax, MAXN)
    nc = _cache[key]

    kvtab = np.concatenate([k, v], axis=1)
    in_maps = []
    for pc in per_core:
        in_maps.append({"kv": kvtab, "qt": q, "meta": pc["meta"],
                        "dstl": pc["dstl"], "dstg": pc["dstg"]})
    res = run_bass_kernel_spmd(nc, in_maps, list(range(NCORES)))

    out = np.zeros((N, HID), np.float32)
    for c, pc in enumerate(per_core):
        n0, n1 = pc["n0"], pc["n1"]
        out[n0:n1] = res.results[c]["xout"][: n1 - n0]
    return out.reshape(1, N, HID)

